# revision 4
# baseline (speedup 1.0000x reference)
"""Fused GAT-masked multi-head attention kernel for Trainium2 (8 NeuronCores).

Problem: B=8, N=1024, DIM=512, 8 heads, 3-layer GraphAttention producing a
[B,N,N] mask that gates the main attention.

Sharding: pure data-parallel over batch — one batch element per core, no
collectives.

Per-core algorithm (all matmuls bf16 with f32 PSUM accumulation; everything
kept in a TRANSPOSED [token-on-partition, row-on-free] layout so that zero
on-device transposes are needed; softmax denominators are computed with
ones-vector matmuls on the TensorEngine since the reduction axis lives on
partitions):

  xT [512,1024], adjT [1024,1024] host-pre-transposed.
  e1/e2 rows   = v_e.T @ xT (weight vectors host-collapsed: gat_W.T@gat_ai)
  per GAT layer l:
    Wh0[m,hid]  = xT.T @ gat_WT          (row form, used as lhsT later)
    eT[m,r]     = leakyrelu(e1[r] + e2[m])          (DVE max(z,.2z))
    expT        = exp(adjT*eT); Sg[r] = ones.T @ expT
    attT        = expT * (1/Sg)[r]                   (softmax, transposed)
    hh[hid,r]   = elu(Wh0.T @ attT + gat_Wb)         (per [128,512] chunk)
    eo1/eo2[r] += w_av.T @ hh                        (Who collapsed away)
  mask stage (att_o / gmask / mask all transposed, exp recomputed instead of
  stored to save SBUF):
    zo = adjT * leakyrelu(eo1[r]+eo2[c]);  So = ones.T@exp(zo)
    att_oT = exp(zo)/So;  Sm = ones.T@exp(att_oT);  maskT = exp(att_oT)/Sm
  attention per head h:
    logitsT[m,r] = (kT slice).T @ (qT*SCALE)        (K=64 matmul)
    expa = exp(logitsT * maskT); S2 = ones.T@expa
    outT[d,r]   += v_rows.T @ expa   (accumulated over m-chunks)
    outT *= (1/S2)[r]
  y[r,f] = sum_h outT[:,h,:].T @ proj_wT + proj_b    (8 x K=64 matmuls)
"""

import numpy as np
import ml_dtypes

import concourse.bass as bass
import concourse.tile as tile
from concourse import bacc, mybir
from concourse.bass_utils import run_bass_kernel_spmd

BF16 = mybir.dt.bfloat16
F32 = mybir.dt.float32
AF = mybir.ActivationFunctionType
OP = mybir.AluOpType

P = 128
N = 1024
DIM = 512
HID = 1024
L = 3
H = 8
HD = 64
SCALE = HD ** -0.5
ALPHA = 0.2
NCH = N // P          # 8 token chunks
CCH = DIM // P        # 4 contraction chunks over DIM
RH = 2                # r halves of 512
F512 = 512

_CACHE = {}


def _bcast_row_ap(row_ap, parts=P):
    """DRAM AP for a [1, F] row read with 0-stride partition broadcast."""
    return bass.AP(tensor=row_ap.tensor, offset=row_ap.offset,
                   ap=[[0, parts]] + list(row_ap.ap)[1:])


def build():
    nc = bacc.Bacc("TRN2", target_bir_lowering=False, debug=False, num_devices=8)

    xT = nc.dram_tensor("xT", [DIM, N], BF16, kind="ExternalInput").ap()
    adjT = nc.dram_tensor("adjT", [N, N], BF16, kind="ExternalInput").ap()
    qkv_wT = nc.dram_tensor("qkv_wT", [DIM, 3 * DIM], BF16, kind="ExternalInput").ap()
    gat_WT = nc.dram_tensor("gat_WT", [DIM, L * HID], BF16, kind="ExternalInput").ap()
    v_e = nc.dram_tensor("v_e", [DIM, 2 * L], BF16, kind="ExternalInput").ap()
    c_e = nc.dram_tensor("c_e", [2 * L, 1], F32, kind="ExternalInput").ap()
    w_av = nc.dram_tensor("w_av", [L * HID, 2], BF16, kind="ExternalInput").ap()
    c_eo = nc.dram_tensor("c_eo", [2, 1], F32, kind="ExternalInput").ap()
    gwb = nc.dram_tensor("gwb", [P, L * NCH], F32, kind="ExternalInput").ap()
    proj_wT2 = nc.dram_tensor("proj_wT2", [HD, H, DIM], BF16, kind="ExternalInput").ap()
    proj_b = nc.dram_tensor("proj_b", [1, DIM], F32, kind="ExternalInput").ap()
    out = nc.dram_tensor("out", [N, DIM], F32, kind="ExternalOutput").ap()

    with tile.TileContext(nc) as tc:
        with tc.tile_pool(name="res", bufs=1) as res, \
             tc.tile_pool(name="dram", bufs=1, space="DRAM") as dram, \
             tc.tile_pool(name="ps_mm", bufs=3, space="PSUM") as ps_mm, \
             tc.tile_pool(name="ps_sum", bufs=2, space="PSUM") as ps_sum:

            # ---------- long-lived tiles ----------
            qT = res.tile([P, H // 2, N], BF16, name="qT")
            kT = res.tile([P, H // 2, N], BF16, name="kT")
            v_sb = res.tile([P, NCH, H, HD], BF16, name="v_sb")
            maskT = res.tile([P, NCH, N], BF16, name="maskT")
            ones_bf = res.tile([P, 1], BF16, name="ones_bf")
            nc.vector.memset(ones_bf, 1.0)
            gwb_sb = res.tile([P, L * NCH], F32, name="gwb_sb")
            nc.sync.dma_start(out=gwb_sb, in_=gwb)
            ce_sb = res.tile([2 * L, 1], F32, name="ce_sb")
            nc.sync.dma_start(out=ce_sb, in_=c_e)
            ceo_sb = res.tile([2, 1], F32, name="ceo_sb")
            nc.sync.dma_start(out=ceo_sb, in_=c_eo)
            pb_b = res.tile([P, DIM], F32, name="pb_b")
            nc.sync.dma_start(out=pb_b, in_=_bcast_row_ap(proj_b))
            w_av_sb = res.tile([P, L * NCH, 2], BF16, name="w_av_sb")
            nc.sync.dma_start(out=w_av_sb,
                              in_=w_av.rearrange("(o p) s -> p o s", p=P))
            v_e_sb = res.tile([P, CCH, 2 * L], BF16, name="v_e_sb")
            nc.sync.dma_start(out=v_e_sb,
                              in_=v_e.rearrange("(o p) s -> p o s", p=P))

            with tc.tile_pool(name="gat", bufs=1) as gp, \
                 tc.tile_pool(name="ps_eo", bufs=2, space="PSUM") as ps_eo:
                xT_sb = gp.tile([P, CCH, N], BF16, name="xT_sb")
                nc.sync.dma_start(out=xT_sb,
                                  in_=xT.rearrange("(o p) r -> p o r", p=P))
                adjT_sb = gp.tile([P, NCH, N], BF16, name="adjT_sb")
                nc.sync.dma_start(out=adjT_sb,
                                  in_=adjT.rearrange("(o p) r -> p o r", p=P))

                # ---------- e1/e2 rows ----------
                e12_sb = gp.tile([2 * L, N], F32, name="e12_sb", tag="row32", bufs=3)
                for half in range(RH):
                    pe = ps_sum.tile([2 * L, F512], F32, name=f"pe_{half}", tag="sum")
                    for c in range(CCH):
                        nc.tensor.matmul(pe, v_e_sb[:, c, :],
                                         xT_sb[:, c, half * F512:(half + 1) * F512],
                                         start=(c == 0), stop=(c == CCH - 1))
                    nc.scalar.copy(e12_sb[:, half * F512:(half + 1) * F512], pe)
                nc.vector.tensor_scalar(e12_sb, e12_sb, ce_sb, None, OP.add)
                e_dram = dram.tile([2 * L, N], F32, name="e_dram")
                nc.sync.dma_start(out=e_dram, in_=e12_sb)

                bcast_e1 = []
                e2col = []
                for l in range(L):
                    b1 = gp.tile([P, N], F32, name=f"bcast_e1_{l}", tag="bc_e1", bufs=2)
                    nc.sync.dma_start(out=b1, in_=_bcast_row_ap(e_dram[2 * l:2 * l + 1, :]))
                    bcast_e1.append(b1)
                    e2c = gp.tile([P, NCH], F32, name=f"e2col_{l}")
                    nc.sync.dma_start(
                        out=e2c,
                        in_=e_dram[2 * l + 1:2 * l + 2, :].rearrange(
                            "one (o p) -> (one p) o", p=P))
                    e2col.append(e2c)

                # eo1/eo2 accumulators live across all layers
                p_eo = [ps_eo.tile([2, F512], F32, name=f"p_eo_{half}", tag="eo")
                        for half in range(RH)]

                # ---------- GAT layers ----------
                for l in range(L):
                    # Wh0 [m, hid] row-form
                    Wh0 = gp.tile([P, NCH, HID], BF16, name=f"Wh0_{l}", tag="big",
                                  bufs=3)
                    for half in range(RH):
                        gw = gp.tile([P, CCH, F512], BF16, name=f"gw_{l}_{half}",
                                     tag="wload", bufs=2)
                        base = l * HID + half * F512
                        nc.sync.dma_start(
                            out=gw,
                            in_=gat_WT[:, base:base + F512].rearrange(
                                "(o p) s -> p o s", p=P))
                        for mt in range(NCH):
                            pm = ps_mm.tile([P, F512], F32, name=f"pWh_{l}_{half}_{mt}",
                                            tag="mm")
                            for c in range(CCH):
                                nc.tensor.matmul(
                                    pm, xT_sb[:, c, mt * P:(mt + 1) * P],
                                    gw[:, c, :],
                                    start=(c == 0), stop=(c == CCH - 1))
                            nc.scalar.copy(
                                Wh0[:, mt, half * F512:(half + 1) * F512], pm)

                    # expT = exp(adjT * leakyrelu(e1[r]+e2[m]))
                    expT = gp.tile([P, NCH, N], BF16, name=f"expT_{l}", tag="big",
                                   bufs=3)
                    psg = [ps_sum.tile([1, F512], F32, name=f"psg_{l}_{h2}", tag="sum")
                           for h2 in range(RH)]
                    for mc in range(NCH):
                        eTc = gp.tile([P, N], F32, name=f"eTc_{l}_{mc}", tag="wf32",
                                      bufs=3)
                        nc.vector.tensor_scalar(eTc, bcast_e1[l],
                                                e2col[l][:, mc:mc + 1], None, OP.add)
                        u = gp.tile([P, N], F32, name=f"u_{l}_{mc}", tag="wf32", bufs=3)
                        nc.vector.tensor_scalar(u, eTc, ALPHA, None, OP.mult)
                        elr = gp.tile([P, N], BF16, name=f"elr_{l}_{mc}", tag="wbf",
                                      bufs=4)
                        nc.vector.tensor_tensor(elr, eTc, u, OP.max)
                        zT = gp.tile([P, N], BF16, name=f"zT_{l}_{mc}", tag="wbf",
                                     bufs=4)
                        nc.vector.tensor_tensor(zT, adjT_sb[:, mc, :], elr, OP.mult)
                        nc.scalar.activation(expT[:, mc, :], zT, AF.Exp)
                        for h2 in range(RH):
                            nc.tensor.matmul(
                                psg[h2], ones_bf,
                                expT[:, mc, h2 * F512:(h2 + 1) * F512],
                                start=(mc == 0), stop=(mc == NCH - 1))

                    sg_sb = gp.tile([1, N], F32, name=f"sg_{l}", tag="row32", bufs=3)
                    for h2 in range(RH):
                        nc.scalar.copy(sg_sb[:, h2 * F512:(h2 + 1) * F512], psg[h2])
                    rsg = gp.tile([1, N], F32, name=f"rsg_{l}", tag="row32", bufs=3)
                    nc.vector.reciprocal(rsg, sg_sb)
                    rsg_bf = gp.tile([1, N], BF16, name=f"rsgb_{l}", tag="rowbf", bufs=2)
                    nc.vector.tensor_copy(rsg_bf, rsg)
                    bcast_rsg = gp.tile([P, N], BF16, name=f"bcrsg_{l}", tag="bcbf",
                                        bufs=2)
                    nc.gpsimd.partition_broadcast(bcast_rsg, rsg_bf)

                    attT = gp.tile([P, NCH, N], BF16, name=f"attT_{l}", tag="big",
                                   bufs=3)
                    for mc in range(NCH):
                        nc.vector.tensor_tensor(attT[:, mc, :], expT[:, mc, :],
                                                bcast_rsg, OP.mult)

                    # hh chunks + eo accumulation
                    for ht in range(NCH):
                        col = gwb_sb[:, l * NCH + ht:l * NCH + ht + 1]
                        for half in range(RH):
                            pm = ps_mm.tile([P, F512], F32, name=f"phh_{l}_{ht}_{half}",
                                            tag="mm")
                            for mc in range(NCH):
                                nc.tensor.matmul(
                                    pm, Wh0[:, mc, ht * P:(ht + 1) * P],
                                    attT[:, mc, half * F512:(half + 1) * F512],
                                    start=(mc == 0), stop=(mc == NCH - 1))
                            zb = gp.tile([P, F512], F32, name=f"zb_{l}_{ht}_{half}",
                                         tag="wh512", bufs=6)
                            nc.vector.tensor_scalar(zb, pm, col, None, OP.add)
                            m0 = gp.tile([P, F512], F32, name=f"m0_{l}_{ht}_{half}",
                                         tag="wh512", bufs=6)
                            nc.vector.tensor_scalar(m0, zb, 0.0, None, OP.min)
                            ex = gp.tile([P, F512], F32, name=f"ex_{l}_{ht}_{half}",
                                         tag="wh512", bufs=6)
                            nc.scalar.activation(ex, m0, AF.Exp)
                            exm1 = gp.tile([P, F512], F32, name=f"exm1_{l}_{ht}_{half}",
                                           tag="wh512", bufs=6)
                            nc.vector.tensor_scalar(exm1, ex, 1.0, None, OP.subtract)
                            hh = gp.tile([P, F512], BF16, name=f"hh_{l}_{ht}_{half}",
                                         tag="hh", bufs=3)
                            nc.vector.tensor_tensor(hh, zb, exm1, OP.max)
                            nc.tensor.matmul(
                                p_eo[half], w_av_sb[:, l * NCH + ht, :], hh,
                                start=(l == 0 and ht == 0),
                                stop=(l == L - 1 and ht == NCH - 1))

                # ---------- qkv (emitted here; scheduler overlaps with mask) ----
                for part, dst, scale in ((0, qT, SCALE), (1, kT, 1.0)):
                    qw = gp.tile([P, CCH, DIM], BF16, name=f"qw_{part}", tag="wload",
                                 bufs=2)
                    nc.sync.dma_start(
                        out=qw,
                        in_=qkv_wT[:, part * DIM:(part + 1) * DIM].rearrange(
                            "(o p) s -> p o s", p=P))
                    for hp in range(H // 2):
                        for half in range(RH):
                            pm = ps_mm.tile([P, F512], F32,
                                            name=f"pqk_{part}_{hp}_{half}", tag="mm")
                            for c in range(CCH):
                                nc.tensor.matmul(
                                    pm, qw[:, c, hp * P:(hp + 1) * P],
                                    xT_sb[:, c, half * F512:(half + 1) * F512],
                                    start=(c == 0), stop=(c == CCH - 1))
                            if scale != 1.0:
                                nc.scalar.mul(
                                    dst[:, hp, half * F512:(half + 1) * F512],
                                    pm, scale)
                            else:
                                nc.scalar.copy(
                                    dst[:, hp, half * F512:(half + 1) * F512], pm)
                vw = gp.tile([P, CCH, DIM], BF16, name="vw", tag="wload", bufs=2)
                nc.sync.dma_start(
                    out=vw,
                    in_=qkv_wT[:, 2 * DIM:3 * DIM].rearrange("(o p) s -> p o s", p=P))
                for mt in range(NCH):
                    pm = ps_mm.tile([P, F512], F32, name=f"pv_{mt}", tag="mm")
                    for c in range(CCH):
                        nc.tensor.matmul(pm, xT_sb[:, c, mt * P:(mt + 1) * P],
                                         vw[:, c, :],
                                         start=(c == 0), stop=(c == CCH - 1))
                    nc.scalar.copy(
                        v_sb[:, mt, :, :].rearrange("p h d -> p (h d)"), pm)

                # ---------- mask stage ----------
                eo12 = gp.tile([2, N], F32, name="eo12", tag="row32", bufs=3)
                for half in range(RH):
                    nc.scalar.copy(eo12[:, half * F512:(half + 1) * F512], p_eo[half])
                nc.vector.tensor_scalar(eo12, eo12, ceo_sb, None, OP.add)
                eo_dram = dram.tile([2, N], F32, name="eo_dram")
                nc.sync.dma_start(out=eo_dram, in_=eo12)
                bcast_eo1 = gp.tile([P, N], F32, name="bcast_eo1", tag="bc_e1", bufs=2)
                nc.sync.dma_start(out=bcast_eo1, in_=_bcast_row_ap(eo_dram[0:1, :]))
                eo2col = gp.tile([P, NCH], F32, name="eo2col")
                nc.sync.dma_start(out=eo2col,
                                  in_=eo_dram[1:2, :].rearrange(
                                      "one (o p) -> (one p) o", p=P))

                zo = gp.tile([P, NCH, N], BF16, name="zo", tag="big", bufs=3)
                pso = [ps_sum.tile([1, F512], F32, name=f"pso_{h2}", tag="sum")
                       for h2 in range(RH)]
                for cc in range(NCH):
                    eTc = gp.tile([P, N], F32, name=f"eoc_{cc}", tag="wf32", bufs=3)
                    nc.vector.tensor_scalar(eTc, bcast_eo1, eo2col[:, cc:cc + 1],
                                            None, OP.add)
                    u = gp.tile([P, N], F32, name=f"uo_{cc}", tag="wf32", bufs=3)
                    nc.vector.tensor_scalar(u, eTc, ALPHA, None, OP.mult)
                    elr = gp.tile([P, N], BF16, name=f"elro_{cc}", tag="wbf", bufs=4)
                    nc.vector.tensor_tensor(elr, eTc, u, OP.max)
                    nc.vector.tensor_tensor(zo[:, cc, :], adjT_sb[:, cc, :], elr,
                                            OP.mult)
                    expoc = gp.tile([P, N], BF16, name=f"expoc_{cc}", tag="wbf",
                                    bufs=4)
                    nc.scalar.activation(expoc, zo[:, cc, :], AF.Exp)
                    for h2 in range(RH):
                        nc.tensor.matmul(pso[h2], ones_bf,
                                         expoc[:, h2 * F512:(h2 + 1) * F512],
                                         start=(cc == 0), stop=(cc == NCH - 1))

                so_sb = gp.tile([1, N], F32, name="so_sb", tag="row32", bufs=3)
                for h2 in range(RH):
                    nc.scalar.copy(so_sb[:, h2 * F512:(h2 + 1) * F512], pso[h2])
                rso = gp.tile([1, N], F32, name="rso", tag="row32", bufs=3)
                nc.vector.reciprocal(rso, so_sb)
                rso_bf = gp.tile([1, N], BF16, name="rso_bf", tag="rowbf", bufs=2)
                nc.vector.tensor_copy(rso_bf, rso)
                bcast_rso = gp.tile([P, N], BF16, name="bcast_rso", tag="bcbf", bufs=2)
                nc.gpsimd.partition_broadcast(bcast_rso, rso_bf)

                aoT = gp.tile([P, NCH, N], BF16, name="aoT", tag="big", bufs=3)
                psm = [ps_sum.tile([1, F512], F32, name=f"psm_{h2}", tag="sum")
                       for h2 in range(RH)]
                for cc in range(NCH):
                    expoc = gp.tile([P, N], BF16, name=f"expo2_{cc}", tag="wbf",
                                    bufs=4)
                    nc.scalar.activation(expoc, zo[:, cc, :], AF.Exp)
                    nc.vector.tensor_tensor(aoT[:, cc, :], expoc, bcast_rso, OP.mult)
                    expmc = gp.tile([P, N], BF16, name=f"expm_{cc}", tag="wbf",
                                    bufs=4)
                    nc.scalar.activation(expmc, aoT[:, cc, :], AF.Exp)
                    for h2 in range(RH):
                        nc.tensor.matmul(psm[h2], ones_bf,
                                         expmc[:, h2 * F512:(h2 + 1) * F512],
                                         start=(cc == 0), stop=(cc == NCH - 1))

                sm_sb = gp.tile([1, N], F32, name="sm_sb", tag="row32", bufs=3)
                for h2 in range(RH):
                    nc.scalar.copy(sm_sb[:, h2 * F512:(h2 + 1) * F512], psm[h2])
                rsm = gp.tile([1, N], F32, name="rsm", tag="row32", bufs=3)
                nc.vector.reciprocal(rsm, sm_sb)
                rsm_bf = gp.tile([1, N], BF16, name="rsm_bf", tag="rowbf", bufs=2)
                nc.vector.tensor_copy(rsm_bf, rsm)
                bcast_rsm = gp.tile([P, N], BF16, name="bcast_rsm", tag="bcbf", bufs=2)
                nc.gpsimd.partition_broadcast(bcast_rsm, rsm_bf)

                for cc in range(NCH):
                    expmc = gp.tile([P, N], BF16, name=f"expm2_{cc}", tag="wbf",
                                    bufs=4)
                    nc.scalar.activation(expmc, aoT[:, cc, :], AF.Exp)
                    nc.vector.tensor_tensor(maskT[:, cc, :], expmc, bcast_rsm,
                                            OP.mult)

            # ---------- attention ----------
            with tc.tile_pool(name="attn", bufs=1) as ap_, \
                 tc.tile_pool(name="ps_out", bufs=2, space="PSUM") as ps_out:
                outT_sb = ap_.tile([HD, H, N], BF16, name="outT_sb")
                projT_sb = ap_.tile([HD, H, DIM], BF16, name="projT_sb")
                nc.sync.dma_start(out=projT_sb, in_=proj_wT2)

                for h in range(H):
                    hp, sub = h // 2, h % 2
                    ps2 = [ps_sum.tile([1, F512], F32, name=f"ps2_{h}_{h2}",
                                       tag="sum") for h2 in range(RH)]
                    po = [ps_out.tile([HD, F512], F32, name=f"po_{h}_{h2}",
                                      tag="out") for h2 in range(RH)]
                    for mc in range(NCH):
                        for h2 in range(RH):
                            pl = ps_mm.tile([P, F512], F32, name=f"pl_{h}_{mc}_{h2}",
                                            tag="mm")
                            nc.tensor.matmul(
                                pl,
                                kT[64 * sub:64 * sub + 64, hp, mc * P:(mc + 1) * P],
                                qT[64 * sub:64 * sub + 64, hp,
                                   h2 * F512:(h2 + 1) * F512],
                                start=True, stop=True)
                            t = ap_.tile([P, F512], F32, name=f"t_{h}_{mc}_{h2}",
                                         tag="t", bufs=3)
                            nc.vector.tensor_tensor(
                                t, pl, maskT[:, mc, h2 * F512:(h2 + 1) * F512],
                                OP.mult)
                            ea = ap_.tile([P, F512], BF16, name=f"ea_{h}_{mc}_{h2}",
                                          tag="ea", bufs=3)
                            nc.scalar.activation(ea, t, AF.Exp)
                            nc.tensor.matmul(ps2[h2], ones_bf, ea,
                                             start=(mc == 0), stop=(mc == NCH - 1))
                            nc.tensor.matmul(po[h2], v_sb[:, mc, h, :], ea,
                                             start=(mc == 0), stop=(mc == NCH - 1))
                    s2_sb = ap_.tile([1, N], F32, name=f"s2_{h}", tag="arow", bufs=3)
                    for h2 in range(RH):
                        nc.scalar.copy(s2_sb[:, h2 * F512:(h2 + 1) * F512], ps2[h2])
                    rs2 = ap_.tile([1, N], F32, name=f"rs2_{h}", tag="arow", bufs=3)
                    nc.vector.reciprocal(rs2, s2_sb)
                    bcast_rs2 = ap_.tile([P, N], F32, name=f"bcrs2_{h}", tag="bcrs2",
                                         bufs=2)
                    nc.gpsimd.partition_broadcast(bcast_rs2, rs2)
                    for h2 in range(RH):
                        nc.vector.tensor_tensor(
                            outT_sb[:, h, h2 * F512:(h2 + 1) * F512], po[h2],
                            bcast_rs2[0:HD, h2 * F512:(h2 + 1) * F512], OP.mult)

                # ---------- final projection ----------
                for rb in range(NCH):
                    py = ps_mm.tile([P, DIM], F32, name=f"py_{rb}", tag="mm")
                    for h in range(H):
                        nc.tensor.matmul(py, outT_sb[:, h, rb * P:(rb + 1) * P],
                                         projT_sb[:, h, :],
                                         start=(h == 0), stop=(h == H - 1))
                    yv = ap_.tile([P, DIM], F32, name=f"yv_{rb}", tag="yv", bufs=3)
                    nc.vector.tensor_tensor(yv, py, pb_b, OP.add)
                    nc.sync.dma_start(out=out[rb * P:(rb + 1) * P, :], in_=yv)

    nc.compile()
    return nc


def _prep_shared(qkv_w, proj_w, proj_b, gat_W, gat_Wb, gat_ai, gat_ai_b,
                 gat_aj, gat_aj_b, out_W, out_Wb, out_ai, out_ai_b,
                 out_aj, out_aj_b):
    bf = ml_dtypes.bfloat16
    f64 = np.float64
    qkv_wT = np.ascontiguousarray(qkv_w.T).astype(bf)
    gat_WT = np.ascontiguousarray(gat_W.transpose(2, 0, 1).reshape(DIM, L * HID)).astype(bf)
    # e1/e2 collapsed weight vectors + constants
    v_e = np.zeros((DIM, 2 * L), f64)
    c_e = np.zeros((2 * L, 1), f64)
    for l in range(L):
        v_e[:, 2 * l] = gat_W[l].astype(f64).T @ gat_ai[l].astype(f64)
        v_e[:, 2 * l + 1] = gat_W[l].astype(f64).T @ gat_aj[l].astype(f64)
        c_e[2 * l, 0] = gat_Wb[l].astype(f64) @ gat_ai[l].astype(f64) + f64(gat_ai_b[l])
        c_e[2 * l + 1, 0] = gat_Wb[l].astype(f64) @ gat_aj[l].astype(f64) + f64(gat_aj_b[l])
    w_ai = out_W.astype(f64).T @ out_ai.astype(f64)
    w_aj = out_W.astype(f64).T @ out_aj.astype(f64)
    w_av = np.stack([w_ai, w_aj], axis=1)
    c_eo = np.array([[out_Wb.astype(f64) @ out_ai.astype(f64) + f64(out_ai_b)],
                     [out_Wb.astype(f64) @ out_aj.astype(f64) + f64(out_aj_b)]])
    gwb = np.ascontiguousarray(
        gat_Wb.reshape(L, NCH, P).transpose(2, 0, 1).reshape(P, L * NCH))
    proj_wT2 = np.ascontiguousarray(
        proj_w.T.reshape(H, HD, DIM).transpose(1, 0, 2)).astype(bf)
    return {
        "qkv_wT": qkv_wT,
        "gat_WT": gat_WT,
        "v_e": v_e.astype(bf),
        "c_e": c_e.astype(np.float32),
        "w_av": w_av.astype(bf),
        "c_eo": c_eo.astype(np.float32),
        "gwb": gwb.astype(np.float32),
        "proj_wT2": proj_wT2,
        "proj_b": np.asarray(proj_b, np.float32).reshape(1, DIM),
    }


def kernel(x, adj, qkv_w, proj_w, proj_b, gat_W, gat_Wb, gat_ai, gat_ai_b,
           gat_aj, gat_aj_b, out_W, out_Wb, out_ai, out_ai_b, out_aj,
           out_aj_b):
    x = np.asarray(x, np.float32)
    adj = np.asarray(adj, np.float32)
    B = x.shape[0]
    assert B == 8 and x.shape[1] == N and x.shape[2] == DIM

    if "nc" not in _CACHE:
        _CACHE["nc"] = build()
    nc = _CACHE["nc"]

    shared = _prep_shared(np.asarray(qkv_w, np.float32),
                          np.asarray(proj_w, np.float32),
                          np.asarray(proj_b, np.float32),
                          np.asarray(gat_W, np.float32),
                          np.asarray(gat_Wb, np.float32),
                          np.asarray(gat_ai, np.float32),
                          np.asarray(gat_ai_b, np.float32),
                          np.asarray(gat_aj, np.float32),
                          np.asarray(gat_aj_b, np.float32),
                          np.asarray(out_W, np.float32),
                          np.asarray(out_Wb, np.float32),
                          np.asarray(out_ai, np.float32),
                          np.asarray(out_ai_b, np.float32),
                          np.asarray(out_aj, np.float32),
                          np.asarray(out_aj_b, np.float32))
    bf = ml_dtypes.bfloat16
    in_maps = []
    for i in range(B):
        m = dict(shared)
        m["xT"] = np.ascontiguousarray(x[i].T).astype(bf)
        m["adjT"] = np.ascontiguousarray(adj[i].T).astype(bf)
        in_maps.append(m)

    res = run_bass_kernel_spmd(nc, in_maps, core_ids=list(range(8)))
    return np.stack([np.asarray(res.results[i]["out"], np.float32)
                     for i in range(B)], axis=0)
